# revision 13
# baseline (speedup 1.0000x reference)
"""Trainium2 Bass kernel for GQA attention (dense_transformer).

Full module: x[1,2048,4096] -> causal GQA attention (32 q heads, 8 kv heads,
head_dim 128, RoPE) -> out[1,2048,4096].

Sharding: tensor-parallel by heads across 8 NeuronCores. Core c owns q heads
4c..4c+3 and kv head c; wq/wk/wv column-sharded, wo row-sharded; x replicated.
The trailing all-reduce over wo partial sums is done host-side (outputs are
gathered to host anyway).

On-chip layout notes:
  - All DRAM-side operands are pre-transposed on host so every matmul operand
    has its contraction dim on SBUF partitions with contiguous DMA patterns.
  - RoPE pairs are de-interleaved host-side (even rows then odd rows of each
    head of wq/wk), which turns the rotation into 64-partition-shifted
    multiply/adds on chip. Dot products are invariant to the permutation.
  - Scores are computed transposed (sk on partitions, sq on free) so the P@V
    matmul needs no on-chip transpose of the probabilities. The softmax
    denominator is accumulated with DVE adds and reduced across partitions
    with a GpSimd partition_all_reduce. Softmax max-subtraction is skipped:
    scores are O(±10) here, exp cannot overflow in fp32, and the result is
    identical up to rounding.
  - Matmuls run as float32r (full PE rate at free-dim >= 256).
"""

import math
from contextlib import ExitStack

import numpy as np

import concourse.bass as bass
import concourse.mybir as mybir
import concourse.tile as tile
from concourse import bacc, bass_isa, bass_utils

F32 = mybir.dt.float32
F32R = mybir.dt.float32r

# Full-scale config (hardcoded; kernel.py must be self-contained).
DIM = 4096
SEQ = 2048
N_HEADS = 32
N_KV_HEADS = 8
HEAD_DIM = 128
N_CORES = 8
HQ = N_HEADS // N_CORES            # q heads per core = 4
CH = 512                           # sq chunk (free dim of most matmuls)
SCALE = 1.0 / math.sqrt(HEAD_DIM)


def build_module(S=SEQ, D=DIM, hq=HQ, ch=CH, use_par_reduce=False):
    """Build the SPMD Bass/Tile module for one core's shard."""
    HD = HEAD_DIM
    H2 = HD // 2
    M = hq * HD                     # local q output dim
    R = ch // 128                   # sk-tiles per sq chunk
    nJ = S // ch                    # sq chunks
    nT = S // 128                   # sk tiles
    nD = D // 128                   # contraction tiles

    nc = bacc.Bacc("TRN2", target_bir_lowering=False, debug=False)
    xT = nc.dram_tensor("xT", [D, S], F32R, kind="ExternalInput").ap()
    wqkvT = nc.dram_tensor("wqkvT", [D, M + 2 * HD], F32R, kind="ExternalInput").ap()
    woT = nc.dram_tensor("woT", [M, D], F32R, kind="ExternalInput").ap()
    constD = nc.dram_tensor("constD", [128, 256], F32R, kind="ExternalInput").ap()
    cosP = nc.dram_tensor("cosP", [HD, S], F32, kind="ExternalInput").ap()
    sinP = nc.dram_tensor("sinP", [HD, S], F32, kind="ExternalInput").ap()
    maskD = nc.dram_tensor("maskD", [128, R * ch], F32, kind="ExternalInput").ap()
    outT = nc.dram_tensor("outT", [D, S], F32, kind="ExternalOutput").ap()

    with tile.TileContext(nc) as tc, ExitStack() as ctx, \
            nc.allow_low_precision(reason="fp32r staging for PE matmuls"):
        Exp = mybir.ActivationFunctionType.Exp

        pers = ctx.enter_context(tc.tile_pool(name="pers", bufs=1))
        qT = [pers.tile([HD, S], F32R, tag=f"qT{h}", name=f"qT{h}") for h in range(hq)]
        kT = pers.tile([HD, S], F32R, tag="kT", name="kT")
        vv = pers.tile([128, nT * HD], F32R, tag="vv", name="vv")
        yT = [pers.tile([HD, S], F32R, tag=f"yT{h}", name=f"yT{h}") for h in range(hq)]
        cosb = pers.tile([HD, S], F32, tag="cosb", name="cosb")
        sinb = pers.tile([HD, S], F32, tag="sinb", name="sinb")
        maskb = pers.tile([128, R * ch], F32, tag="maskb", name="maskb")
        ident = pers.tile([128, 128], F32R, tag="ident", name="ident")
        ones_col = pers.tile([128, 1], F32R, tag="ones_col", name="ones_col")
        ones_row = pers.tile([1, 128], F32R, tag="ones_row", name="ones_row")
        nc.sync.dma_start(cosb[:], cosP[:])
        nc.sync.dma_start(sinb[:], sinP[:])
        nc.sync.dma_start(maskb[:], maskD[:])
        nc.sync.dma_start(ident[:], constD[:, 0:128])
        nc.sync.dma_start(ones_col[:], constD[:, 128:129])
        nc.sync.dma_start(ones_row[:], constD[0:1, 128:256])

        rpool = ctx.enter_context(tc.tile_pool(name="rpool", bufs=2))

        def rope(out, ps, j):
            """out[:,chunk] = RoPE(ps) with de-interleaved halves.

            The 64-partition swap always pairs a PSUM operand with an SBUF
            operand (mixed-space ops may differ in base partition; SB+SB
            ops must not)."""
            cj = cosb[:, j * ch:(j + 1) * ch]
            sj = sinb[:, j * ch:(j + 1) * ch]
            nc.vector.tensor_mul(out, ps[:], cj)
            tmp = rpool.tile([HD, ch], F32, tag="ropetmp", name="ropetmp")
            nc.vector.tensor_mul(tmp[0:H2, :], ps[H2:HD, :], sj[0:H2, :])
            nc.vector.tensor_mul(tmp[H2:HD, :], ps[0:H2, :], sj[H2:HD, :])
            nc.vector.tensor_add(out, out, tmp[:])

        # ---- Phase 1: QKV projections (+RoPE, +v transpose) ----
        wpool = ctx.enter_context(tc.tile_pool(name="wpool", bufs=3))
        xpool = ctx.enter_context(tc.tile_pool(name="xpool", bufs=3))
        vpool = ctx.enter_context(tc.tile_pool(name="vpool", bufs=2))
        with tc.tile_pool(name="qkv_ps", bufs=1, space="PSUM") as qkv_ps, \
             tc.tile_pool(name="vt_ps", bufs=2, space="PSUM") as vt_ps:
            for j in range(nJ):
                ps_q = [qkv_ps.tile([HD, ch], F32, tag=f"psq{m}", name=f"psq{m}")
                        for m in range(hq)]
                ps_k = qkv_ps.tile([HD, ch], F32, tag="psk", name="psk")
                ps_v = qkv_ps.tile([HD, ch], F32, tag="psv", name="psv")
                for d in range(nD):
                    wt = wpool.tile([128, M + 2 * HD], F32R, tag="wt", name="wt")
                    nc.sync.dma_start(wt[:], wqkvT[d * 128:(d + 1) * 128, :])
                    xt = xpool.tile([128, ch], F32R, tag="xt", name="xt")
                    nc.sync.dma_start(
                        xt[:], xT[d * 128:(d + 1) * 128, j * ch:(j + 1) * ch])
                    st, sp = (d == 0), (d == nD - 1)
                    xr = xt[:]
                    for m in range(hq):
                        nc.tensor.matmul(
                            ps_q[m][:], wt[:, m * HD:(m + 1) * HD],
                            xr, start=st, stop=sp)
                    nc.tensor.matmul(
                        ps_k[:], wt[:, M:M + HD], xr,
                        start=st, stop=sp)
                    nc.tensor.matmul(
                        ps_v[:], wt[:, M + HD:M + 2 * HD], xr,
                        start=st, stop=sp)
                for m in range(hq):
                    rope(qT[m][:, j * ch:(j + 1) * ch], ps_q[m], j)
                rope(kT[:, j * ch:(j + 1) * ch], ps_k, j)
                # v: psum [hd, ch] -> sbuf, then PE-transpose per 128 block
                vt_s = vpool.tile([HD, ch], F32R, tag="vts", name="vts")
                nc.scalar.copy(vt_s[:], ps_v[:])
                for r in range(R):
                    t = j * R + r
                    pvt = vt_ps.tile([128, 128], F32R, tag="pvt", name="pvt")
                    nc.tensor.transpose(
                        pvt[:], vt_s[:, r * 128:(r + 1) * 128], ident[:])
                    nc.scalar.copy(vv[:, t * HD:(t + 1) * HD], pvt[:])

        # ---- Phase 2: attention (transposed flash-style, causal) ----
        apool = ctx.enter_context(tc.tile_pool(name="apool", bufs=4))
        npool = ctx.enter_context(tc.tile_pool(name="npool", bufs=2))
        with tc.tile_pool(name="attn_ps", bufs=2, space="PSUM") as attn_ps:
            for h in range(hq):
                for j in range(nJ):
                    nTj = (j + 1) * R   # causal sk-tile count for this chunk
                    y_ps = attn_ps.tile([HD, ch], F32, tag="yps", name="yps")
                    acc = npool.tile([128, ch], F32R, tag="acc", name="acc")
                    qslice = qT[h][:, j * ch:(j + 1) * ch]
                    for t in range(nTj):
                        s_ps = attn_ps.tile([128, ch], F32, tag="sps", name="sps")
                        nc.tensor.matmul(
                            s_ps[:], kT[:, t * 128:(t + 1) * 128],
                            qslice, start=True, stop=True)
                        et = apool.tile([128, ch], F32R, tag="exp", name="et")
                        nc.scalar.activation(et[:], s_ps[:], Exp, scale=SCALE)
                        r = t - j * R
                        if r >= 0:  # diagonal tile: apply causal mask
                            nc.vector.tensor_mul(
                                et[:], et[:], maskb[:, r * ch:(r + 1) * ch])
                        if t == 0:
                            nc.vector.tensor_copy(acc[:], et[:])
                        else:
                            nc.vector.tensor_add(acc[:], acc[:], et[:])
                        nc.tensor.matmul(
                            y_ps[:], vv[:, t * HD:(t + 1) * HD],
                            et[:],
                            start=(t == 0), stop=(t == nTj - 1))
                    if use_par_reduce:
                        den = npool.tile([128, ch], F32, tag="den", name="den")
                        nc.gpsimd.partition_all_reduce(
                            den[:], acc[:], 128, bass_isa.ReduceOp.add)
                        rec = npool.tile([128, ch], F32, tag="rec", name="rec")
                        nc.vector.reciprocal(rec[:], den[:])
                    else:
                        # denominator: column-sum of acc via ones-matmul,
                        # reciprocal, then PE broadcast to all partitions.
                        ps_d = attn_ps.tile([1, ch], F32, tag="dps", name="dps",
                                            bufs=1)
                        nc.tensor.matmul(ps_d[:], ones_col[:],
                                         acc[:],
                                         start=True, stop=True)
                        rec1 = npool.tile([1, ch], F32R, tag="rec1", name="rec1")
                        nc.vector.reciprocal(rec1[:], ps_d[:])
                        ps_b = attn_ps.tile([128, ch], F32, tag="bps",
                                            name="bps", bufs=1)
                        nc.tensor.matmul(ps_b[:], ones_row[:],
                                         rec1[:],
                                         start=True, stop=True)
                        rec = npool.tile([128, ch], F32, tag="rec", name="rec")
                        nc.vector.tensor_copy(rec[:], ps_b[:])
                    nc.vector.tensor_mul(
                        yT[h][:, j * ch:(j + 1) * ch], y_ps[:], rec[:])

        # ---- Phase 3: output projection (row-parallel wo partial sums) ----
        opool = ctx.enter_context(tc.tile_pool(name="opool", bufs=3))
        wopool = ctx.enter_context(tc.tile_pool(name="wopool", bufs=4))
        with tc.tile_pool(name="wo_ps", bufs=1, space="PSUM") as wo_ps:
            for dt in range(nD):
                ps_o = [wo_ps.tile([128, ch], F32, tag=f"pso{j}", name=f"pso{j}")
                        for j in range(nJ)]
                for o in range(hq):
                    wot = wopool.tile([128, 128], F32R, tag="wot", name="wot")
                    nc.sync.dma_start(
                        wot[:], woT[o * 128:(o + 1) * 128,
                                    dt * 128:(dt + 1) * 128])
                    for j in range(nJ):
                        nc.tensor.matmul(
                            ps_o[j][:], wot[:],
                            yT[o][:, j * ch:(j + 1) * ch],
                            start=(o == 0), stop=(o == hq - 1))
                for j in range(nJ):
                    ot = opool.tile([128, ch], F32, tag="osb", name="osb")
                    nc.scalar.copy(ot[:], ps_o[j][:])
                    nc.sync.dma_start(
                        outT[dt * 128:(dt + 1) * 128, j * ch:(j + 1) * ch],
                        ot[:])
    nc.compile()
    return nc


def _deinterleave_perm(hd):
    """Row permutation putting even indices first, odd second."""
    return np.concatenate([np.arange(0, hd, 2), np.arange(1, hd, 2)])


def host_prep(x, wq, wk, wv, wo, freqs_cos, freqs_sin,
              n_cores=N_CORES, hq=HQ, n_kv=N_KV_HEADS):
    """Build the per-core input maps (numpy, host-side)."""
    HD = HEAD_DIM
    D = x.shape[-1]
    S = x.shape[-2]
    M = hq * HD
    R = CH // 128
    x = np.asarray(x, np.float32).reshape(S, D)
    wq = np.asarray(wq, np.float32)
    wk = np.asarray(wk, np.float32)
    wv = np.asarray(wv, np.float32)
    wo = np.asarray(wo, np.float32)
    fc = np.asarray(freqs_cos, np.float32)
    fs = np.asarray(freqs_sin, np.float32)

    perm = _deinterleave_perm(HD)
    xT = np.ascontiguousarray(x.T)                      # [D, S]
    cosP = np.ascontiguousarray(np.concatenate([fc.T, fc.T], 0))  # [128, S]
    sinP = np.ascontiguousarray(np.concatenate([-fs.T, fs.T], 0))
    # mask[t, r*CH + s] = 1 if 128*r + t <= s else 0
    tt = np.arange(128)[:, None]
    ss = np.arange(CH)[None, :]
    maskD = np.concatenate(
        [(128 * r + tt <= ss).astype(np.float32) for r in range(R)], axis=1)
    maskD = np.ascontiguousarray(maskD)                 # [128, R*CH]
    constD = np.concatenate(
        [np.eye(128, dtype=np.float32), np.ones((128, 128), np.float32)],
        axis=1)                                         # [128, 256]

    in_maps = []
    for c in range(n_cores):
        wq_c = wq[c * M:(c + 1) * M, :].reshape(hq, HD, D)[:, perm, :]
        wq_c = wq_c.reshape(M, D)
        wk_c = wk[c * HD:(c + 1) * HD, :][perm, :]
        wv_c = wv[c * HD:(c + 1) * HD, :]
        wqkvT = np.ascontiguousarray(
            np.concatenate([wq_c, wk_c, wv_c], axis=0).T)  # [D, M+256]
        woT = np.ascontiguousarray(wo[:, c * M:(c + 1) * M].T)  # [M, D]
        in_maps.append({
            "xT": xT, "wqkvT": wqkvT, "woT": woT, "constD": constD,
            "cosP": cosP, "sinP": sinP, "maskD": maskD,
        })
    return in_maps


_NC_CACHE = {}


def _get_module():
    if "nc" not in _NC_CACHE:
        _NC_CACHE["nc"] = build_module()
    return _NC_CACHE["nc"]


def run_on_cores(in_maps, trace=False):
    nc = _get_module()
    res = bass_utils.run_bass_kernel_spmd(
        nc, in_maps, core_ids=list(range(len(in_maps))), trace=trace)
    return res


def kernel(x, wq, wk, wv, wo, freqs_cos, freqs_sin):
    in_maps = host_prep(x, wq, wk, wv, wo, freqs_cos, freqs_sin)
    res = run_on_cores(in_maps)
    acc = None
    for r in res.results:
        o = r["outT"]
        acc = o.astype(np.float64) if acc is None else acc + o
    out = acc.T.astype(np.float32).reshape(1, SEQ, DIM)
    return out


# revision 18
# speedup vs baseline: 1.1293x; 1.1293x over previous
"""Trainium2 Bass kernel for GQA attention (dense_transformer).

Full module: x[1,2048,4096] -> causal GQA attention (32 q heads, 8 kv heads,
head_dim 128, RoPE) -> out[1,2048,4096].

Sharding: tensor-parallel by heads across 8 NeuronCores. Core c owns q heads
4c..4c+3 and kv head c; wq/wk/wv column-sharded, wo row-sharded; x replicated.
The trailing all-reduce over wo partial sums is done host-side (outputs are
gathered to host anyway).

On-chip layout notes:
  - All DRAM-side operands are pre-transposed on host so every matmul operand
    has its contraction dim on SBUF partitions with contiguous DMA patterns.
  - RoPE pairs are de-interleaved host-side (even rows then odd rows of each
    head of wq/wk), which turns the rotation into 64-partition-shifted
    multiply/adds on chip. Dot products are invariant to the permutation.
  - Scores are computed transposed (sk on partitions, sq on free) so the P@V
    matmul needs no on-chip transpose of the probabilities. The softmax
    denominator is accumulated with DVE adds and reduced across partitions
    with a GpSimd partition_all_reduce. Softmax max-subtraction is skipped:
    scores are O(±10) here, exp cannot overflow in fp32, and the result is
    identical up to rounding.
  - Matmuls run as float32r (full PE rate at free-dim >= 256).
"""

import math
from contextlib import ExitStack

import numpy as np

import concourse.bass as bass
import concourse.mybir as mybir
import concourse.tile as tile
from concourse import bacc, bass_isa, bass_utils

F32 = mybir.dt.float32
F32R = mybir.dt.float32r

# Full-scale config (hardcoded; kernel.py must be self-contained).
DIM = 4096
SEQ = 2048
N_HEADS = 32
N_KV_HEADS = 8
HEAD_DIM = 128
N_CORES = 8
HQ = N_HEADS // N_CORES            # q heads per core = 4
CH = 512                           # sq chunk (free dim of most matmuls)
SCALE = 1.0 / math.sqrt(HEAD_DIM)


def build_module(S=SEQ, D=DIM, hq=HQ, ch=CH, use_par_reduce=False):
    """Build the SPMD Bass/Tile module for one core's shard."""
    HD = HEAD_DIM
    H2 = HD // 2
    M = hq * HD                     # local q output dim
    R = ch // 128                   # sk-tiles per sq chunk
    nJ = S // ch                    # sq chunks
    nT = S // 128                   # sk tiles
    nD = D // 128                   # contraction tiles

    nc = bacc.Bacc("TRN2", target_bir_lowering=False, debug=False)
    xT = nc.dram_tensor("xT", [D, S], F32R, kind="ExternalInput").ap()
    wqkvT = nc.dram_tensor("wqkvT", [D, M + 2 * HD], F32R, kind="ExternalInput").ap()
    woT = nc.dram_tensor("woT", [M, D], F32R, kind="ExternalInput").ap()
    constD = nc.dram_tensor("constD", [128, 256], F32R, kind="ExternalInput").ap()
    cosP = nc.dram_tensor("cosP", [HD, S], F32, kind="ExternalInput").ap()
    sinP = nc.dram_tensor("sinP", [HD, S], F32, kind="ExternalInput").ap()
    maskD = nc.dram_tensor("maskD", [128, R * ch], F32, kind="ExternalInput").ap()
    outT = nc.dram_tensor("outT", [D, S], F32, kind="ExternalOutput").ap()

    with tile.TileContext(nc) as tc, ExitStack() as ctx, \
            nc.allow_low_precision(reason="fp32r staging for PE matmuls"):
        Exp = mybir.ActivationFunctionType.Exp

        pers = ctx.enter_context(tc.tile_pool(name="pers", bufs=1))
        qT = [pers.tile([HD, S], F32R, tag=f"qT{h}", name=f"qT{h}") for h in range(hq)]
        kT = pers.tile([HD, S], F32R, tag="kT", name="kT")
        vv = pers.tile([128, nT * HD], F32R, tag="vv", name="vv")
        ident = pers.tile([128, 128], F32R, tag="ident", name="ident")
        ones_col = pers.tile([128, 1], F32R, tag="ones_col", name="ones_col")
        ones_row = pers.tile([1, 128], F32R, tag="ones_row", name="ones_row")
        nc.sync.dma_start(ident[:], constD[:, 0:128])
        nc.sync.dma_start(ones_col[:], constD[:, 128:129])
        nc.sync.dma_start(ones_row[:], constD[0:1, 128:256])

        rpool = ctx.enter_context(tc.tile_pool(name="rpool", bufs=2))

        def rope(out, ps, j):
            """out[:,chunk] = RoPE(ps) with de-interleaved halves.

            The 64-partition swap always pairs a PSUM operand with an SBUF
            operand (mixed-space ops may differ in base partition; SB+SB
            ops must not)."""
            cj = cosb[:, j * ch:(j + 1) * ch]
            sj = sinb[:, j * ch:(j + 1) * ch]
            nc.vector.tensor_mul(out, ps[:], cj)
            tmp = rpool.tile([HD, ch], F32, tag="ropetmp", name="ropetmp")
            nc.vector.tensor_mul(tmp[0:H2, :], ps[H2:HD, :], sj[0:H2, :])
            nc.vector.tensor_mul(tmp[H2:HD, :], ps[0:H2, :], sj[H2:HD, :])
            nc.vector.tensor_add(out, out, tmp[:])

        # ---- Phase 1: QKV projections (+RoPE, +v transpose) ----
        # All QKV weights preloaded once (12 MB resident for this phase);
        # re-reading them per sq-chunk made v1 DMA-bound.
        xpool = ctx.enter_context(tc.tile_pool(name="xpool", bufs=4))
        vpool = ctx.enter_context(tc.tile_pool(name="vpool", bufs=2))
        MW = M + 2 * HD
        wqkv_r = wqkvT.rearrange("(d p) m -> p d m", p=128)
        with tc.tile_pool(name="wpool", bufs=1) as wpool, \
             tc.tile_pool(name="qkv_ps", bufs=1, space="PSUM") as qkv_ps, \
             tc.tile_pool(name="vt_ps", bufs=2, space="PSUM") as vt_ps:
            wsb = wpool.tile([128, nD, MW], F32R, tag="wsb", name="wsb")
            for d in range(nD):
                nc.sync.dma_start(wsb[:, d, :], wqkv_r[:, d, :])
            cosb = wpool.tile([HD, S], F32, tag="cosb", name="cosb")
            sinb = wpool.tile([HD, S], F32, tag="sinb", name="sinb")
            nc.sync.dma_start(cosb[:], cosP[:])
            nc.sync.dma_start(sinb[:], sinP[:])
            for j in range(nJ):
                ps_q = [qkv_ps.tile([HD, ch], F32, tag=f"psq{m}", name=f"psq{m}")
                        for m in range(hq)]
                ps_k = qkv_ps.tile([HD, ch], F32, tag="psk", name="psk")
                ps_v = qkv_ps.tile([HD, ch], F32, tag="psv", name="psv")
                for d in range(nD):
                    xt = xpool.tile([128, ch], F32R, tag="xt", name="xt")
                    nc.sync.dma_start(
                        xt[:], xT[d * 128:(d + 1) * 128, j * ch:(j + 1) * ch])
                    st, sp = (d == 0), (d == nD - 1)
                    xr = xt[:]
                    wt = wsb[:, d, :]
                    for m in range(hq):
                        nc.tensor.matmul(
                            ps_q[m][:], wt[:, m * HD:(m + 1) * HD],
                            xr, start=st, stop=sp)
                    nc.tensor.matmul(
                        ps_k[:], wt[:, M:M + HD], xr,
                        start=st, stop=sp)
                    nc.tensor.matmul(
                        ps_v[:], wt[:, M + HD:M + 2 * HD], xr,
                        start=st, stop=sp)
                for m in range(hq):
                    rope(qT[m][:, j * ch:(j + 1) * ch], ps_q[m], j)
                rope(kT[:, j * ch:(j + 1) * ch], ps_k, j)
                # v: psum [hd, ch] -> sbuf, then PE-transpose per 128 block
                vt_s = vpool.tile([HD, ch], F32R, tag="vts", name="vts")
                nc.vector.tensor_copy(vt_s[:], ps_v[:])
                for r in range(R):
                    t = j * R + r
                    pvt = vt_ps.tile([128, 128], F32R, tag="pvt", name="pvt")
                    nc.tensor.transpose(
                        pvt[:], vt_s[:, r * 128:(r + 1) * 128], ident[:])
                    nc.vector.tensor_copy(vv[:, t * HD:(t + 1) * HD], pvt[:])

        # ---- Phases 2+3 share the yT/mask pool (opened after weights free) ----
        ypool = ctx.enter_context(tc.tile_pool(name="ypool", bufs=1))
        yT = [ypool.tile([HD, S], F32R, tag=f"yT{h}", name=f"yT{h}")
              for h in range(hq)]
        maskb = ypool.tile([128, R * ch], F32, tag="maskb", name="maskb")
        nc.sync.dma_start(maskb[:], maskD[:])

        # ---- Phase 2: attention (transposed flash-style, causal) ----
        apool = ctx.enter_context(tc.tile_pool(name="apool", bufs=4))
        npool = ctx.enter_context(tc.tile_pool(name="npool", bufs=2))
        with tc.tile_pool(name="attn_ps", bufs=2, space="PSUM") as attn_ps:
            for h in range(hq):
                for j in range(nJ):
                    nTj = (j + 1) * R   # causal sk-tile count for this chunk
                    y_ps = attn_ps.tile([HD, ch], F32, tag="yps", name="yps")
                    acc = npool.tile([128, ch], F32R, tag="acc", name="acc")
                    qslice = qT[h][:, j * ch:(j + 1) * ch]
                    for t in range(nTj):
                        s_ps = attn_ps.tile([128, ch], F32, tag="sps", name="sps")
                        nc.tensor.matmul(
                            s_ps[:], kT[:, t * 128:(t + 1) * 128],
                            qslice, start=True, stop=True)
                        et = apool.tile([128, ch], F32R, tag="exp", name="et")
                        # scale folded into wq host-side; ACT does pure exp
                        nc.scalar.activation(et[:], s_ps[:], Exp)
                        r = t - j * R
                        if r >= 0:  # diagonal tile: apply causal mask
                            nc.vector.tensor_mul(
                                et[:], et[:], maskb[:, r * ch:(r + 1) * ch])
                        if t == 0:
                            nc.vector.tensor_copy(acc[:], et[:])
                        else:
                            nc.vector.tensor_add(acc[:], acc[:], et[:])
                        nc.tensor.matmul(
                            y_ps[:], vv[:, t * HD:(t + 1) * HD],
                            et[:],
                            start=(t == 0), stop=(t == nTj - 1))
                    if use_par_reduce:
                        den = npool.tile([128, ch], F32, tag="den", name="den")
                        nc.gpsimd.partition_all_reduce(
                            den[:], acc[:], 128, bass_isa.ReduceOp.add)
                        rec = npool.tile([128, ch], F32, tag="rec", name="rec")
                        nc.vector.reciprocal(rec[:], den[:])
                    else:
                        # denominator: column-sum of acc via ones-matmul,
                        # reciprocal, then PE broadcast to all partitions.
                        ps_d = attn_ps.tile([1, ch], F32, tag="dps", name="dps",
                                            bufs=1)
                        nc.tensor.matmul(ps_d[:], ones_col[:],
                                         acc[:],
                                         start=True, stop=True)
                        rec1 = npool.tile([1, ch], F32R, tag="rec1", name="rec1")
                        nc.vector.reciprocal(rec1[:], ps_d[:])
                        ps_b = attn_ps.tile([128, ch], F32, tag="bps",
                                            name="bps", bufs=1)
                        nc.tensor.matmul(ps_b[:], ones_row[:],
                                         rec1[:],
                                         start=True, stop=True)
                        rec = npool.tile([128, ch], F32, tag="rec", name="rec")
                        nc.vector.tensor_copy(rec[:], ps_b[:])
                    nc.vector.tensor_mul(
                        yT[h][:, j * ch:(j + 1) * ch], y_ps[:], rec[:])

        # ---- Phase 3: output projection (row-parallel wo partial sums) ----
        opool = ctx.enter_context(tc.tile_pool(name="opool", bufs=3))
        wopool = ctx.enter_context(tc.tile_pool(name="wopool", bufs=4))
        with tc.tile_pool(name="wo_ps", bufs=1, space="PSUM") as wo_ps:
            for dt in range(nD):
                ps_o = [wo_ps.tile([128, ch], F32, tag=f"pso{j}", name=f"pso{j}")
                        for j in range(nJ)]
                for o in range(hq):
                    wot = wopool.tile([128, 128], F32R, tag="wot", name="wot")
                    nc.sync.dma_start(
                        wot[:], woT[o * 128:(o + 1) * 128,
                                    dt * 128:(dt + 1) * 128])
                    for j in range(nJ):
                        nc.tensor.matmul(
                            ps_o[j][:], wot[:],
                            yT[o][:, j * ch:(j + 1) * ch],
                            start=(o == 0), stop=(o == hq - 1))
                for j in range(nJ):
                    ot = opool.tile([128, ch], F32, tag="osb", name="osb")
                    nc.vector.tensor_copy(ot[:], ps_o[j][:])
                    nc.sync.dma_start(
                        outT[dt * 128:(dt + 1) * 128, j * ch:(j + 1) * ch],
                        ot[:])
    nc.compile()
    return nc


def _deinterleave_perm(hd):
    """Row permutation putting even indices first, odd second."""
    return np.concatenate([np.arange(0, hd, 2), np.arange(1, hd, 2)])


def host_prep(x, wq, wk, wv, wo, freqs_cos, freqs_sin,
              n_cores=N_CORES, hq=HQ, n_kv=N_KV_HEADS):
    """Build the per-core input maps (numpy, host-side)."""
    HD = HEAD_DIM
    D = x.shape[-1]
    S = x.shape[-2]
    M = hq * HD
    R = CH // 128
    x = np.asarray(x, np.float32).reshape(S, D)
    wq = np.asarray(wq, np.float32)
    wk = np.asarray(wk, np.float32)
    wv = np.asarray(wv, np.float32)
    wo = np.asarray(wo, np.float32)
    fc = np.asarray(freqs_cos, np.float32)
    fs = np.asarray(freqs_sin, np.float32)

    perm = _deinterleave_perm(HD)
    wq = wq * np.float32(SCALE)   # fold softmax scale into q projection
    xT = np.ascontiguousarray(x.T)                      # [D, S]
    cosP = np.ascontiguousarray(np.concatenate([fc.T, fc.T], 0))  # [128, S]
    sinP = np.ascontiguousarray(np.concatenate([-fs.T, fs.T], 0))
    # mask[t, r*CH + s] = 1 if 128*r + t <= s else 0
    tt = np.arange(128)[:, None]
    ss = np.arange(CH)[None, :]
    maskD = np.concatenate(
        [(128 * r + tt <= ss).astype(np.float32) for r in range(R)], axis=1)
    maskD = np.ascontiguousarray(maskD)                 # [128, R*CH]
    constD = np.concatenate(
        [np.eye(128, dtype=np.float32), np.ones((128, 128), np.float32)],
        axis=1)                                         # [128, 256]

    in_maps = []
    for c in range(n_cores):
        wq_c = wq[c * M:(c + 1) * M, :].reshape(hq, HD, D)[:, perm, :]
        wq_c = wq_c.reshape(M, D)
        wk_c = wk[c * HD:(c + 1) * HD, :][perm, :]
        wv_c = wv[c * HD:(c + 1) * HD, :]
        wqkvT = np.ascontiguousarray(
            np.concatenate([wq_c, wk_c, wv_c], axis=0).T)  # [D, M+256]
        woT = np.ascontiguousarray(wo[:, c * M:(c + 1) * M].T)  # [M, D]
        in_maps.append({
            "xT": xT, "wqkvT": wqkvT, "woT": woT, "constD": constD,
            "cosP": cosP, "sinP": sinP, "maskD": maskD,
        })
    return in_maps


_NC_CACHE = {}


def _get_module():
    if "nc" not in _NC_CACHE:
        _NC_CACHE["nc"] = build_module()
    return _NC_CACHE["nc"]


def run_on_cores(in_maps, trace=False):
    nc = _get_module()
    res = bass_utils.run_bass_kernel_spmd(
        nc, in_maps, core_ids=list(range(len(in_maps))), trace=trace)
    return res


def kernel(x, wq, wk, wv, wo, freqs_cos, freqs_sin):
    in_maps = host_prep(x, wq, wk, wv, wo, freqs_cos, freqs_sin)
    res = run_on_cores(in_maps)
    acc = None
    for r in res.results:
        o = r["outT"]
        acc = o.astype(np.float64) if acc is None else acc + o
    out = acc.T.astype(np.float32).reshape(1, SEQ, DIM)
    return out


# revision 20
# speedup vs baseline: 1.2396x; 1.0977x over previous
"""Trainium2 Bass kernel for GQA attention (dense_transformer).

Full module: x[1,2048,4096] -> causal GQA attention (32 q heads, 8 kv heads,
head_dim 128, RoPE) -> out[1,2048,4096].

Sharding: tensor-parallel by heads across 8 NeuronCores. Core c owns q heads
4c..4c+3 and kv head c; wq/wk/wv column-sharded, wo row-sharded; x replicated.
The trailing all-reduce over wo partial sums is done host-side (outputs are
gathered to host anyway).

On-chip layout notes:
  - All DRAM-side operands are pre-transposed on host so every matmul operand
    has its contraction dim on SBUF partitions with contiguous DMA patterns.
  - RoPE pairs are de-interleaved host-side (even rows then odd rows of each
    head of wq/wk), which turns the rotation into 64-partition-shifted
    multiply/adds on chip. Dot products are invariant to the permutation.
  - Scores are computed transposed (sk on partitions, sq on free) so the P@V
    matmul needs no on-chip transpose of the probabilities. The softmax
    denominator is accumulated with DVE adds and reduced across partitions
    with a GpSimd partition_all_reduce. Softmax max-subtraction is skipped:
    scores are O(±10) here, exp cannot overflow in fp32, and the result is
    identical up to rounding.
  - Matmuls run as float32r (full PE rate at free-dim >= 256).
"""

import math
from contextlib import ExitStack

import numpy as np

import concourse.bass as bass
import concourse.mybir as mybir
import concourse.tile as tile
from concourse import bacc, bass_isa, bass_utils

F32 = mybir.dt.float32
F32R = mybir.dt.float32r

# Full-scale config (hardcoded; kernel.py must be self-contained).
DIM = 4096
SEQ = 2048
N_HEADS = 32
N_KV_HEADS = 8
HEAD_DIM = 128
N_CORES = 8
HQ = N_HEADS // N_CORES            # q heads per core = 4
CH = 512                           # sq chunk (free dim of most matmuls)
SCALE = 1.0 / math.sqrt(HEAD_DIM)


def build_module(S=SEQ, D=DIM, hq=HQ, ch=CH, use_par_reduce=False):
    """Build the SPMD Bass/Tile module for one core's shard."""
    HD = HEAD_DIM
    H2 = HD // 2
    M = hq * HD                     # local q output dim
    R = ch // 128                   # sk-tiles per sq chunk
    nJ = S // ch                    # sq chunks
    nT = S // 128                   # sk tiles
    nD = D // 128                   # contraction tiles

    nc = bacc.Bacc("TRN2", target_bir_lowering=False, debug=False)
    xT = nc.dram_tensor("xT", [D, S], F32R, kind="ExternalInput").ap()
    wqkvT = nc.dram_tensor("wqkvT", [D, M + 2 * HD], F32R, kind="ExternalInput").ap()
    woT = nc.dram_tensor("woT", [M, D], F32R, kind="ExternalInput").ap()
    constD = nc.dram_tensor("constD", [128, 256], F32R, kind="ExternalInput").ap()
    cosP = nc.dram_tensor("cosP", [HD, S], F32, kind="ExternalInput").ap()
    sinP = nc.dram_tensor("sinP", [HD, S], F32, kind="ExternalInput").ap()
    maskD = nc.dram_tensor("maskD", [128, R * ch], F32, kind="ExternalInput").ap()
    outT = nc.dram_tensor("outT", [D, S], F32, kind="ExternalOutput").ap()

    with tile.TileContext(nc) as tc, ExitStack() as ctx, \
            nc.allow_low_precision(reason="fp32r staging for PE matmuls"):
        Exp = mybir.ActivationFunctionType.Exp

        pers = ctx.enter_context(tc.tile_pool(name="pers", bufs=1))
        qT = [pers.tile([HD, S], F32R, tag=f"qT{h}", name=f"qT{h}") for h in range(hq)]
        kT = pers.tile([HD, S], F32R, tag="kT", name="kT")
        vv = pers.tile([128, nT * HD], F32R, tag="vv", name="vv")
        ident = pers.tile([128, 128], F32R, tag="ident", name="ident")
        ones_col = pers.tile([128, 1], F32R, tag="ones_col", name="ones_col")
        ones_row = pers.tile([1, 128], F32R, tag="ones_row", name="ones_row")
        nc.sync.dma_start(ident[:], constD[:, 0:128])
        nc.sync.dma_start(ones_col[:], constD[:, 128:129])
        nc.sync.dma_start(ones_row[:], constD[0:1, 128:256])

        rpool = ctx.enter_context(tc.tile_pool(name="rpool", bufs=2))

        def rope(out, ps, j):
            """out[:,chunk] = RoPE(ps) with de-interleaved halves.

            The 64-partition swap always pairs a PSUM operand with an SBUF
            operand (mixed-space ops may differ in base partition; SB+SB
            ops must not)."""
            cj = cosb[:, j * ch:(j + 1) * ch]
            sj = sinb[:, j * ch:(j + 1) * ch]
            nc.vector.tensor_mul(out, ps[:], cj)
            tmp = rpool.tile([HD, ch], F32, tag="ropetmp", name="ropetmp")
            nc.vector.tensor_mul(tmp[0:H2, :], ps[H2:HD, :], sj[0:H2, :])
            nc.vector.tensor_mul(tmp[H2:HD, :], ps[0:H2, :], sj[H2:HD, :])
            nc.vector.tensor_add(out, out, tmp[:])

        # ---- Phase 1: QKV projections (+RoPE, +v transpose) ----
        # All QKV weights preloaded once (12 MB resident for this phase);
        # re-reading them per sq-chunk made v1 DMA-bound.
        xpool = ctx.enter_context(tc.tile_pool(name="xpool", bufs=4))
        vpool = ctx.enter_context(tc.tile_pool(name="vpool", bufs=2))
        MW = M + 2 * HD
        wqkv_r = wqkvT.rearrange("(d p) m -> p d m", p=128)
        with tc.tile_pool(name="wpool", bufs=1) as wpool, \
             tc.tile_pool(name="qkv_ps", bufs=1, space="PSUM") as qkv_ps, \
             tc.tile_pool(name="vt_ps", bufs=2, space="PSUM") as vt_ps:
            wsb = wpool.tile([128, nD, MW], F32R, tag="wsb", name="wsb")
            for d in range(nD):
                nc.sync.dma_start(wsb[:, d, :], wqkv_r[:, d, :])
            cosb = wpool.tile([HD, S], F32, tag="cosb", name="cosb")
            sinb = wpool.tile([HD, S], F32, tag="sinb", name="sinb")
            nc.sync.dma_start(cosb[:], cosP[:])
            nc.sync.dma_start(sinb[:], sinP[:])
            for j in range(nJ):
                ps_q = [qkv_ps.tile([HD, ch], F32, tag=f"psq{m}", name=f"psq{m}")
                        for m in range(hq)]
                ps_k = qkv_ps.tile([HD, ch], F32, tag="psk", name="psk")
                ps_v = qkv_ps.tile([HD, ch], F32, tag="psv", name="psv")
                for d in range(nD):
                    xt = xpool.tile([128, ch], F32R, tag="xt", name="xt")
                    nc.sync.dma_start(
                        xt[:], xT[d * 128:(d + 1) * 128, j * ch:(j + 1) * ch])
                    st, sp = (d == 0), (d == nD - 1)
                    xr = xt[:]
                    wt = wsb[:, d, :]
                    for m in range(hq):
                        nc.tensor.matmul(
                            ps_q[m][:], wt[:, m * HD:(m + 1) * HD],
                            xr, start=st, stop=sp)
                    nc.tensor.matmul(
                        ps_k[:], wt[:, M:M + HD], xr,
                        start=st, stop=sp)
                    nc.tensor.matmul(
                        ps_v[:], wt[:, M + HD:M + 2 * HD], xr,
                        start=st, stop=sp)
                for m in range(hq):
                    rope(qT[m][:, j * ch:(j + 1) * ch], ps_q[m], j)
                rope(kT[:, j * ch:(j + 1) * ch], ps_k, j)
                # v: psum [hd, ch] -> sbuf, then PE-transpose per 128 block
                vt_s = vpool.tile([HD, ch], F32R, tag="vts", name="vts")
                nc.vector.tensor_copy(vt_s[:], ps_v[:])
                for r in range(R):
                    t = j * R + r
                    pvt = vt_ps.tile([128, 128], F32R, tag="pvt", name="pvt")
                    nc.tensor.transpose(
                        pvt[:], vt_s[:, r * 128:(r + 1) * 128], ident[:])
                    nc.vector.tensor_copy(vv[:, t * HD:(t + 1) * HD], pvt[:])

        # ---- Phases 2+3 share the yT/mask pool (opened after weights free) ----
        ypool = ctx.enter_context(tc.tile_pool(name="ypool", bufs=1))
        yT = [ypool.tile([HD, S], F32R, tag=f"yT{h}", name=f"yT{h}")
              for h in range(hq)]
        maskb = ypool.tile([128, R * ch], F32, tag="maskb", name="maskb")
        nc.sync.dma_start(maskb[:], maskD[:])

        # ---- Phase 2: attention (transposed flash-style, causal) ----
        apool = ctx.enter_context(tc.tile_pool(name="apool", bufs=4))
        npool = ctx.enter_context(tc.tile_pool(name="npool", bufs=2))
        with tc.tile_pool(name="attn_ps", bufs=2, space="PSUM") as attn_ps:
            for h in range(hq):
                for j in range(nJ):
                    nTj = (j + 1) * R   # causal sk-tile count for this chunk
                    y_ps = attn_ps.tile([HD, ch], F32, tag="yps", name="yps")
                    ps_d = attn_ps.tile([1, ch], F32, tag="dps", name="dps",
                                        bufs=2)
                    qslice = qT[h][:, j * ch:(j + 1) * ch]
                    for t in range(nTj):
                        s_ps = attn_ps.tile([128, ch], F32, tag="sps", name="sps")
                        nc.tensor.matmul(
                            s_ps[:], kT[:, t * 128:(t + 1) * 128],
                            qslice, start=True, stop=True)
                        et = apool.tile([128, ch], F32R, tag="exp", name="et")
                        # scale folded into wq host-side; ACT does pure exp
                        nc.scalar.activation(et[:], s_ps[:], Exp)
                        r = t - j * R
                        if r >= 0:  # diagonal tile: apply causal mask
                            nc.vector.tensor_mul(
                                et[:], et[:], maskb[:, r * ch:(r + 1) * ch])
                        # softmax denominator accumulates on PE (ones-matmul)
                        nc.tensor.matmul(
                            ps_d[:], ones_col[:], et[:],
                            start=(t == 0), stop=(t == nTj - 1))
                        nc.tensor.matmul(
                            y_ps[:], vv[:, t * HD:(t + 1) * HD],
                            et[:],
                            start=(t == 0), stop=(t == nTj - 1))
                    rec1 = npool.tile([1, ch], F32R, tag="rec1", name="rec1")
                    nc.vector.reciprocal(rec1[:], ps_d[:])
                    ps_b = attn_ps.tile([128, ch], F32, tag="bps",
                                        name="bps", bufs=2)
                    nc.tensor.matmul(ps_b[:], ones_row[:], rec1[:],
                                     start=True, stop=True)
                    rec = npool.tile([128, ch], F32, tag="rec", name="rec")
                    nc.vector.tensor_copy(rec[:], ps_b[:])
                    nc.vector.tensor_mul(
                        yT[h][:, j * ch:(j + 1) * ch], y_ps[:], rec[:])

        # ---- Phase 3: output projection (row-parallel wo partial sums) ----
        opool = ctx.enter_context(tc.tile_pool(name="opool", bufs=3))
        wopool = ctx.enter_context(tc.tile_pool(name="wopool", bufs=4))
        with tc.tile_pool(name="wo_ps", bufs=1, space="PSUM") as wo_ps:
            for dt in range(nD):
                ps_o = [wo_ps.tile([128, ch], F32, tag=f"pso{j}", name=f"pso{j}",
                                   bufs=2)
                        for j in range(nJ)]
                for o in range(hq):
                    wot = wopool.tile([128, 128], F32R, tag="wot", name="wot")
                    nc.sync.dma_start(
                        wot[:], woT[o * 128:(o + 1) * 128,
                                    dt * 128:(dt + 1) * 128])
                    for j in range(nJ):
                        nc.tensor.matmul(
                            ps_o[j][:], wot[:],
                            yT[o][:, j * ch:(j + 1) * ch],
                            start=(o == 0), stop=(o == hq - 1))
                for j in range(nJ):
                    ot = opool.tile([128, ch], F32, tag="osb", name="osb")
                    nc.vector.tensor_copy(ot[:], ps_o[j][:])
                    nc.sync.dma_start(
                        outT[dt * 128:(dt + 1) * 128, j * ch:(j + 1) * ch],
                        ot[:])
    nc.compile()
    return nc


def _deinterleave_perm(hd):
    """Row permutation putting even indices first, odd second."""
    return np.concatenate([np.arange(0, hd, 2), np.arange(1, hd, 2)])


def host_prep(x, wq, wk, wv, wo, freqs_cos, freqs_sin,
              n_cores=N_CORES, hq=HQ, n_kv=N_KV_HEADS):
    """Build the per-core input maps (numpy, host-side)."""
    HD = HEAD_DIM
    D = x.shape[-1]
    S = x.shape[-2]
    M = hq * HD
    R = CH // 128
    x = np.asarray(x, np.float32).reshape(S, D)
    wq = np.asarray(wq, np.float32)
    wk = np.asarray(wk, np.float32)
    wv = np.asarray(wv, np.float32)
    wo = np.asarray(wo, np.float32)
    fc = np.asarray(freqs_cos, np.float32)
    fs = np.asarray(freqs_sin, np.float32)

    perm = _deinterleave_perm(HD)
    wq = wq * np.float32(SCALE)   # fold softmax scale into q projection
    xT = np.ascontiguousarray(x.T)                      # [D, S]
    cosP = np.ascontiguousarray(np.concatenate([fc.T, fc.T], 0))  # [128, S]
    sinP = np.ascontiguousarray(np.concatenate([-fs.T, fs.T], 0))
    # mask[t, r*CH + s] = 1 if 128*r + t <= s else 0
    tt = np.arange(128)[:, None]
    ss = np.arange(CH)[None, :]
    maskD = np.concatenate(
        [(128 * r + tt <= ss).astype(np.float32) for r in range(R)], axis=1)
    maskD = np.ascontiguousarray(maskD)                 # [128, R*CH]
    constD = np.concatenate(
        [np.eye(128, dtype=np.float32), np.ones((128, 128), np.float32)],
        axis=1)                                         # [128, 256]

    in_maps = []
    for c in range(n_cores):
        wq_c = wq[c * M:(c + 1) * M, :].reshape(hq, HD, D)[:, perm, :]
        wq_c = wq_c.reshape(M, D)
        wk_c = wk[c * HD:(c + 1) * HD, :][perm, :]
        wv_c = wv[c * HD:(c + 1) * HD, :]
        wqkvT = np.ascontiguousarray(
            np.concatenate([wq_c, wk_c, wv_c], axis=0).T)  # [D, M+256]
        woT = np.ascontiguousarray(wo[:, c * M:(c + 1) * M].T)  # [M, D]
        in_maps.append({
            "xT": xT, "wqkvT": wqkvT, "woT": woT, "constD": constD,
            "cosP": cosP, "sinP": sinP, "maskD": maskD,
        })
    return in_maps


_NC_CACHE = {}


def _get_module():
    if "nc" not in _NC_CACHE:
        _NC_CACHE["nc"] = build_module()
    return _NC_CACHE["nc"]


def run_on_cores(in_maps, trace=False):
    nc = _get_module()
    res = bass_utils.run_bass_kernel_spmd(
        nc, in_maps, core_ids=list(range(len(in_maps))), trace=trace)
    return res


def kernel(x, wq, wk, wv, wo, freqs_cos, freqs_sin):
    in_maps = host_prep(x, wq, wk, wv, wo, freqs_cos, freqs_sin)
    res = run_on_cores(in_maps)
    acc = None
    for r in res.results:
        o = r["outT"]
        acc = o.astype(np.float64) if acc is None else acc + o
    out = acc.T.astype(np.float32).reshape(1, SEQ, DIM)
    return out
